# revision 16
# baseline (speedup 1.0000x reference)
"""Self-contained Trainium2 Bass kernel for nn_BigramLanguageModel.

Sharding: data-parallel over batch (8 rows -> 8 NeuronCores), weights
replicated, no collectives. Each core runs the full 12-layer transformer on
its [1024, 512] slice in a transposed layout xT [C, T] (channels on
partitions): every matmul then uses a natural weight layout as lhsT and no PE
transposes are needed. LN affine params are folded into adjacent weight
matrices on the host (exact here since gains are 1 / biases 0).

Matmuls run in fp32r (TF32, ~11-bit mantissa, 4x fp32 throughput) by default;
the fp32 residual stream and fp32 PSUM accumulation keep error ~1e-3.
Set BASS_NO_F32R=1 for full-fp32 matmuls.

Attention per head-group of 4 heads: row-packed K=32 score matmuls
(tile_position), max-free exp (scores are O(1) bounded), causal handled by
chunk clipping + a -30000 additive mask on the diagonal 128x128 blocks,
unnormalized exp(S)@V col-packed plus an M=32 ones-matmul for replicated
denominators, one reciprocal+multiply at the end.
"""
import os
import ml_dtypes
import numpy as np

import concourse.bass as bass
import concourse.bacc as bacc
import concourse.mybir as mybir
import concourse.tile as tile
from concourse.bass_utils import run_bass_kernel_spmd

F32 = mybir.dt.float32
F32R = mybir.dt.float32r
BF16 = mybir.dt.bfloat16
AF = mybir.ActivationFunctionType
ALU = mybir.AluOpType

V, LAB, C, T_MAX, H, L = 8192, 64, 512, 1024, 16, 12
HS = C // H            # 32
FF = 4 * C             # 2048
B, T = 8, 1024
P = 128
KT = C // P            # 4 k-tiles over C
MT = 4                 # 4 m-tiles of output channels
NCH = T // 512         # 2 chunks of 512 tokens
EPS = 1e-5
MASKNEG = -30000.0

USE_F32R = os.environ.get("BASS_NO_F32R", "") == ""
N_LAYERS = int(os.environ.get("BASS_KERNEL_LAYERS", str(L)))
DT_W = F32R if USE_F32R else F32
DT_ATT = BF16 if USE_F32R else F32   # PV/sums operands (fp32r dst-pattern ban)


def _pb(ap, parts=P):
    """Partition-broadcast a [1, ...] AP to `parts` partitions (step 0)."""
    return bass.AP(tensor=ap.tensor, offset=ap.offset,
                   ap=[[0, parts]] + [list(x) for x in ap.ap[1:]])


def build_program(n_layers=N_LAYERS):
    nc = bacc.Bacc("TRN2", target_bir_lowering=False, debug=False)

    # ---- DRAM IO ----  (DT_W tensors are host-prerounded to tf32 when f32r)
    x0t = nc.dram_tensor("x0t", [C, T], F32, kind="ExternalInput")
    wqkv = nc.dram_tensor("wqkv", [n_layers, 3, C, C], DT_W, kind="ExternalInput")
    wo = nc.dram_tensor("wo", [n_layers, C, C], DT_W, kind="ExternalInput")
    w1 = nc.dram_tensor("w1", [n_layers, C, FF], DT_W, kind="ExternalInput")
    w2 = nc.dram_tensor("w2", [n_layers, FF, C], DT_W, kind="ExternalInput")
    # per-partition bias blob [n_layers, 128, 36]:
    # cols 0-3 bq, 4-7 bk, 12-15 bo, 16-31 b1, 32-35 b2 (col j = channel 128j+p)
    bias = nc.dram_tensor("bias", [n_layers, P, 36], F32, kind="ExternalInput")
    bvrow = nc.dram_tensor("bvrow", [n_layers, C], DT_W, kind="ExternalInput")
    maskd = nc.dram_tensor("maskd", [P, P], F32, kind="ExternalInput")  # 0 / -30000
    ones_d = nc.dram_tensor("ones_d", [P, 32], DT_W, kind="ExternalInput")
    ones16_d = nc.dram_tensor("ones16_d", [P, 32], DT_ATT, kind="ExternalInput")
    onesr_d = nc.dram_tensor("onesr_d", [1, P], DT_W, kind="ExternalInput")
    wlm = nc.dram_tensor("wlm", [C, V], DT_W, kind="ExternalInput")
    blmrow = nc.dram_tensor("blmrow", [V], DT_W, kind="ExternalInput")

    logits = nc.dram_tensor("logits", [T, V], F32, kind="ExternalOutput")
    lse = nc.dram_tensor("lse", [T, 1], F32, kind="ExternalOutput")

    import contextlib
    with tile.TileContext(nc) as tc, contextlib.ExitStack() as main, \
            nc.allow_low_precision(reason="deliberate tf32 (fp32r) matmul pipeline"):
        cst = main.enter_context(tc.tile_pool(name="cst", bufs=1))
        actp = main.enter_context(tc.tile_pool(name="actp", bufs=1))
        smallp = main.enter_context(tc.tile_pool(name="smallp", bufs=1))
        statp = main.enter_context(tc.tile_pool(name="statp", bufs=1))
        psum = main.enter_context(tc.tile_pool(name="psum", bufs=1, space="PSUM"))

        # constants
        mask_sb = cst.tile([P, P], F32)
        nc.sync.dma_start(out=mask_sb, in_=maskd[:, :])
        ones32 = cst.tile([P, 32], DT_W)
        nc.sync.dma_start(out=ones32, in_=ones_d[:, :])
        ones_bf = cst.tile([P, 32], DT_ATT)
        nc.sync.dma_start(out=ones_bf, in_=ones16_d[:, :])
        ones1f = cst.tile([P, 1], F32)
        nc.vector.memset(ones1f, 1.0)
        eps_sb = cst.tile([1, 1], F32)
        nc.vector.memset(eps_sb, EPS)
        onesrow = cst.tile([1, P], DT_W)
        nc.sync.dma_start(out=onesrow, in_=onesr_d[:, :])

        # persistent activations (x stays exact fp32; z is the tf32 point)
        x_sb = actp.tile([P, KT, T], F32, name="x_sb")
        nc.sync.dma_start(out=x_sb, in_=x0t.rearrange("(m p) t -> p m t", p=P))

        def layernorm(src, dst):
            """LN over C: src [P, KT, T] fp32, dst [P, KT, T] DT_W.

            Partition sums via ones-matmul; mean/rstd broadcast back to all
            128 partitions with a K=1 ones-matmul into PSUM (engines cannot
            read partition-broadcast APs).
            """
            mean = statp.tile([1, T], DT_W, tag="st_mean")
            rstd = statp.tile([1, T], DT_W, tag="st_rstd")
            var = statp.tile([1, T], F32, tag="st_var")
            for c in range(NCH):
                cs = slice(512 * c, 512 * (c + 1))
                m1 = psum.tile([1, 512], F32, tag="acc1", bufs=2)
                m2 = psum.tile([1, 512], F32, tag="acc2", bufs=2)
                for k in range(KT):     # plain fp32 matmul (x is fp32)
                    nc.tensor.matmul(m1, ones1f, src[:, k, cs],
                                     start=(k == 0), stop=(k == KT - 1))
                for k in range(KT):     # squares rounded to DT_W by ACT
                    sq = smallp.tile([P, 512], DT_W, tag="sq", bufs=2)
                    nc.scalar.activation(sq, src[:, k, cs], AF.Square)
                    nc.tensor.matmul(m2, ones32[:, 0:1], sq,
                                     start=(k == 0), stop=(k == KT - 1))
                nc.scalar.activation(mean[:, cs], m1, AF.Copy, scale=1.0 / C)
                nc.scalar.activation(var[:, cs], m2, AF.Copy, scale=1.0 / C)
            nc.vector.tensor_mul(rstd, mean, mean)
            nc.vector.tensor_sub(var, var, rstd)
            nc.scalar.activation(var, var, AF.Sqrt, bias=eps_sb)
            nc.vector.reciprocal(out=rstd, in_=var)
            for c in range(NCH):
                cs = slice(512 * c, 512 * (c + 1))
                mrep = psum.tile([P, 512], F32, tag="acc1", bufs=2)
                rrep = psum.tile([P, 512], F32, tag="acc2", bufs=2)
                nc.tensor.matmul(mrep, onesrow, mean[:, cs], start=True, stop=True)
                nc.tensor.matmul(rrep, onesrow, rstd[:, cs], start=True, stop=True)
                for k in range(KT):
                    nc.vector.tensor_sub(dst[:, k, cs], src[:, k, cs], mrep)
                    nc.vector.tensor_mul(dst[:, k, cs], dst[:, k, cs], rrep)

        z_sb = actp.tile([P, KT, T], DT_W, name="z_sb")

        layer_stack = contextlib.ExitStack()
        actp2 = layer_stack.enter_context(tc.tile_pool(name="actp2", bufs=1))
        qt_sb = actp2.tile([P, MT, T], DT_W, name="qt_sb")
        kt_sb = actp2.tile([P, MT, T], DT_W, name="kt_sb")
        v_sb = actp2.tile([P, T // P, C], DT_ATT, name="v_sb")
        ot_sb = actp2.tile([P, MT, T], DT_W, name="ot_sb")

        wqkvp = layer_stack.enter_context(tc.tile_pool(name="wqkvp", bufs=1))
        wop = layer_stack.enter_context(tc.tile_pool(name="wop", bufs=1))
        w1p = layer_stack.enter_context(tc.tile_pool(name="w1p", bufs=1))
        w2p = layer_stack.enter_context(tc.tile_pool(name="w2p", bufs=1))
        biasp = layer_stack.enter_context(tc.tile_pool(name="biasp", bufs=1))
        expp = layer_stack.enter_context(tc.tile_pool(name="expp", bufs=1))

        for l in range(n_layers):
            wqkv_sb = wqkvp.tile([P, 3, KT, C], DT_W, tag="wqkv")
            nc.sync.dma_start(out=wqkv_sb,
                              in_=wqkv[l].rearrange("w (k p) n -> p w k n", p=P))
            wo_sb = wop.tile([P, KT, C], DT_W, tag="wo")
            nc.sync.dma_start(out=wo_sb,
                              in_=wo[l].rearrange("(k p) n -> p k n", p=P))
            bias_sb = biasp.tile([P, 36], F32, tag="bias")
            nc.sync.dma_start(out=bias_sb, in_=bias[l])
            bv_sb = biasp.tile([1, C], DT_W, tag="bvrow")
            nc.sync.dma_start(out=bv_sb, in_=bvrow[l][None, :])

            # ---- LN1 ----
            layernorm(x_sb, z_sb)

            # ---- q/k projections -> qt_sb/kt_sb [P, MT, T] (ch-major) ----
            for proj, (dst, bcol) in enumerate(((qt_sb, 0), (kt_sb, 4))):
                for c in range(NCH):
                    cs = slice(512 * c, 512 * (c + 1))
                    ps = psum.tile([P, MT, 512], F32, tag="big4")
                    for m in range(MT):
                        for k in range(KT):
                            nc.tensor.matmul(
                                ps[:, m, :],
                                wqkv_sb[:, proj, k, P * m:P * (m + 1)],
                                z_sb[:, k, cs],
                                start=(k == 0), stop=(k == KT - 1))
                    for m in range(MT):
                        nc.scalar.activation(dst[:, m, cs], ps[:, m, :],
                                             AF.Identity,
                                             bias=bias_sb[:, bcol + m:bcol + m + 1])

            # ---- v projection -> v_sb [P, 8, C] ([token, ch] layout) ----
            for s in range(T // P):
                ps = psum.tile([P, 512], F32, tag="acc1", bufs=2)
                for k in range(KT):
                    nc.tensor.matmul(ps, z_sb[:, k, P * s:P * (s + 1)],
                                     wqkv_sb[:, 2, k, :],
                                     start=(k == 0), stop=False)
                nc.tensor.matmul(ps, onesrow, bv_sb, start=False, stop=True)
                nc.scalar.activation(v_sb[:, s, :], ps, AF.Copy)

            # ---- attention: 4 head-groups x 2 chunks, causal ----
            for g in range(MT):
                for c in range(NCH):
                    c0 = 512 * c
                    pv = psum.tile([P, 512], F32, tag="acc1", bufs=2)
                    sm = psum.tile([P, 512], F32, tag="acc2", bufs=2)
                    ns = (c + 1) * 4          # s-tiles with s0 < chunk end
                    for s in range(ns):
                        s0 = P * s
                        t_lo = max(s0, c0)
                        w0 = t_lo - c0        # col offset within chunk
                        sc = psum.tile([P, 4, 512], F32, tag="big4")
                        for h in range(4):
                            nc.tensor.matmul(
                                sc[:, h, w0:512],
                                kt_sb[32 * h:32 * h + 32, g, s0:s0 + P],
                                qt_sb[32 * h:32 * h + 32, g, t_lo:c0 + 512],
                                start=True, stop=True,
                                tile_position=(32 * h, 0))
                        if s0 >= c0:   # diagonal block: additive causal mask
                            for h in range(4):
                                nc.vector.tensor_add(
                                    out=sc[:, h, w0:w0 + P],
                                    in0=sc[:, h, w0:w0 + P],
                                    in1=mask_sb)
                        et = expp.tile([P, 4, 512], DT_ATT, tag="expT")
                        nc.scalar.activation(et[:, :, w0:512], sc[:, :, w0:512],
                                             AF.Exp)
                        for h in range(4):
                            hh = 32 * (4 * g + h)
                            nc.tensor.matmul(
                                pv[32 * h:32 * h + 32, w0:512],
                                v_sb[:, s, hh:hh + 32],
                                et[:, h, w0:512],
                                start=(s == 0), stop=(s == ns - 1),
                                tile_position=(0, 32 * h),
                                skip_group_check=True)
                        for h in range(4):
                            nc.tensor.matmul(
                                sm[32 * h:32 * h + 32, w0:512],
                                ones_bf,
                                et[:, h, w0:512],
                                start=(s == 0), stop=(s == ns - 1),
                                tile_position=(0, 32 * h),
                                skip_group_check=True)
                    rs = smallp.tile([P, 512], F32, tag="rsum", bufs=1)
                    nc.vector.reciprocal(out=rs, in_=sm)
                    nc.vector.tensor_mul(ot_sb[:, g, c0:c0 + 512], pv, rs)

            # ---- output projection + residual ----
            for c in range(NCH):
                cs = slice(512 * c, 512 * (c + 1))
                ps = psum.tile([P, MT, 512], F32, tag="big4")
                for m in range(MT):
                    for k in range(KT):
                        nc.tensor.matmul(ps[:, m, :],
                                         wo_sb[:, k, P * m:P * (m + 1)],
                                         ot_sb[:, k, cs],
                                         start=(k == 0), stop=(k == KT - 1))
                for m in range(MT):
                    nc.vector.scalar_tensor_tensor(
                        out=x_sb[:, m, cs], in0=ps[:, m, :],
                        scalar=bias_sb[:, 12 + m:13 + m], in1=x_sb[:, m, cs],
                        op0=ALU.add, op1=ALU.add)

            # ---- LN2 + FFN (two f-halves, each residual-accumulated) ----
            layernorm(x_sb, z_sb)
            for quarter in range(4):
                w1_sb = w1p.tile([P, KT, 512], DT_W, tag="w1")
                nc.sync.dma_start(
                    out=w1_sb,
                    in_=w1[l].rearrange("(k p) f -> p k f", p=P)
                    [:, :, 512 * quarter:512 * (quarter + 1)])
                w2_sb = w2p.tile([P, 4, C], DT_W, tag="w2")
                nc.sync.dma_start(
                    out=w2_sb,
                    in_=w2[l, 512 * quarter:512 * (quarter + 1), :]
                    .rearrange("(f p) n -> p f n", p=P))
                for c in range(NCH):
                    cs = slice(512 * c, 512 * (c + 1))
                    fps = psum.tile([P, MT, 512], F32, tag="big4")
                    for fi in range(4):
                        hp = psum.tile([P, 512], F32, tag="acc1", bufs=2)
                        for k in range(KT):
                            nc.tensor.matmul(hp,
                                             w1_sb[:, k, P * fi:P * (fi + 1)],
                                             z_sb[:, k, cs],
                                             start=(k == 0), stop=(k == KT - 1))
                        h1 = smallp.tile([P, 512], DT_W, tag="h1", bufs=2)
                        bcol = 16 + 4 * quarter + fi
                        nc.scalar.activation(h1, hp, AF.Relu,
                                             bias=bias_sb[:, bcol:bcol + 1])
                        for m in range(MT):
                            nc.tensor.matmul(fps[:, m, :],
                                             w2_sb[:, fi, P * m:P * (m + 1)],
                                             h1,
                                             start=(fi == 0), stop=(fi == 3))
                    for m in range(MT):
                        nc.vector.scalar_tensor_tensor(
                            out=x_sb[:, m, cs], in0=fps[:, m, :],
                            scalar=(bias_sb[:, 32 + m:33 + m] if quarter == 0 else 0.0),
                            in1=x_sb[:, m, cs],
                            op0=ALU.add, op1=ALU.add)

        # ---- final LN + LM head + lse ----
        layernorm(x_sb, z_sb)
        layer_stack.close()
        wlmp = main.enter_context(tc.tile_pool(name="wlmp", bufs=3))
        lmoutp = main.enter_context(tc.tile_pool(name="lmoutp", bufs=1))
        for t in range(T // P):
            t0 = P * t
            partials = lmoutp.tile([P, 16], F32, tag="partials", bufs=2)
            for vc in range(V // 512):
                wlm_sb = wlmp.tile([P, KT, 512], DT_W, tag="wlm")
                nc.sync.dma_start(
                    out=wlm_sb,
                    in_=wlm.rearrange("(k p) n -> p k n", p=P)
                    [:, :, 512 * vc:512 * (vc + 1)])
                blm_sb = wlmp.tile([1, 512], DT_W, tag="blm")
                nc.sync.dma_start(out=blm_sb,
                                  in_=blmrow[None, 512 * vc:512 * (vc + 1)])
                ps = psum.tile([P, 512], F32, tag="acc1", bufs=2)
                for k in range(KT):
                    nc.tensor.matmul(ps, z_sb[:, k, t0:t0 + P],
                                     wlm_sb[:, k, :],
                                     start=(k == 0), stop=False)
                nc.tensor.matmul(ps, onesrow, blm_sb, start=False, stop=True)
                lg = lmoutp.tile([P, 512], F32, tag="lgchunk", bufs=3)
                nc.scalar.activation(lg, ps, AF.Copy)
                nc.sync.dma_start(out=logits[t0:t0 + P, 512 * vc:512 * (vc + 1)],
                                  in_=lg)
                etmp = lmoutp.tile([P, 512], F32, tag="etmp", bufs=2)
                nc.scalar.activation(etmp, ps, AF.Exp,
                                     accum_out=partials[:, vc:vc + 1])
            lse_sb = lmoutp.tile([P, 1], F32, tag="lse", bufs=2)
            nc.vector.reduce_sum(out=lse_sb, in_=partials,
                                 axis=mybir.AxisListType.X)
            nc.scalar.activation(lse_sb, lse_sb, AF.Ln)
            nc.sync.dma_start(out=lse[t0:t0 + P, :], in_=lse_sb)

    nc.finalize()
    return nc


# ------------------------- host side -------------------------

def _tf32(a):
    """Round-to-nearest tf32 (11-bit mantissa), matching walrus fp32_to_fp32r."""
    if not USE_F32R:
        return np.ascontiguousarray(a)
    u = np.ascontiguousarray(a).view(np.uint32)
    r = (u.astype(np.uint64) + 0x7FF + ((u >> 12) & 1)).astype(np.uint32) & np.uint32(0xFFFFF000)
    return r.view(np.float32)


def prep_arrays(inputs, n_layers=N_LAYERS):
    f = lambda k: np.ascontiguousarray(np.asarray(inputs[k], dtype=np.float32))
    scale = np.float32(HS ** -0.5)
    Wq_r = f('Wq').transpose(0, 2, 1, 3).reshape(L, C, C)   # [L, C(in), H*HS]
    Wk_r = f('Wk').transpose(0, 2, 1, 3).reshape(L, C, C)
    Wv_r = f('Wv').transpose(0, 2, 1, 3).reshape(L, C, C)
    g1 = f('ln1_g')[:, :, None]
    b1v = f('ln1_b')
    wqkv = np.stack([Wq_r * g1 * scale, Wk_r * g1, Wv_r * g1], axis=1)  # [L,3,C,C]
    bq = np.einsum('lc,lcn->ln', b1v, Wq_r) * scale
    bk = np.einsum('lc,lcn->ln', b1v, Wk_r)
    bv = np.einsum('lc,lcn->ln', b1v, Wv_r)
    g2 = f('ln2_g')[:, :, None]
    b2v = f('ln2_b')
    w1 = f('W1') * g2
    b1_eff = f('b1') + np.einsum('lc,lcf->lf', b2v, f('W1'))
    wlm = f('Wlm') * f('lnf_g')[:, None]
    blm = f('blm') + f('lnf_b') @ f('Wlm')

    # bias blob [L, 128, 36]: col j of a group maps channel 128j+p
    bias = np.zeros((L, P, 36), np.float32)
    bias[:, :, 0:4] = bq.reshape(L, 4, P).transpose(0, 2, 1)
    bias[:, :, 4:8] = bk.reshape(L, 4, P).transpose(0, 2, 1)
    bias[:, :, 12:16] = f('bo').reshape(L, 4, P).transpose(0, 2, 1)
    bias[:, :, 16:32] = b1_eff.reshape(L, 16, P).transpose(0, 2, 1)
    bias[:, :, 32:36] = f('b2').reshape(L, 4, P).transpose(0, 2, 1)

    # additive causal mask for diagonal blocks: 0 where t >= s else -30000
    maskd = np.where(np.triu(np.ones((P, P), bool)), 0.0, MASKNEG).astype(np.float32)

    shared = dict(
        wqkv=_tf32(wqkv[:n_layers]),
        wo=_tf32(f('Wo')[:n_layers]),
        w1=_tf32(w1[:n_layers]),
        w2=_tf32(f('W2')[:n_layers]),
        bias=np.ascontiguousarray(bias[:n_layers]),
        bvrow=_tf32(bv[:n_layers]),
        maskd=maskd,
        ones_d=np.ones((P, 32), np.float32),
        ones16_d=np.ones((P, 32), ml_dtypes.bfloat16 if USE_F32R else np.float32),
        onesr_d=np.ones((1, P), np.float32),
        wlm=_tf32(wlm),
        blmrow=_tf32(blm),
    )

    tok_emb, lab_emb, pos_emb = f('tok_emb'), f('lab_emb'), f('pos_emb')
    idx = np.asarray(inputs['idx'])
    idl = np.asarray(inputs['idx_labels'])
    x0 = tok_emb[idx] + lab_emb[idl] + pos_emb[None, :T]     # [B, T, C]
    x0t = np.ascontiguousarray(x0.transpose(0, 2, 1))        # [B, C, T]
    return shared, x0t


_CACHED = {}


def _get_program():
    key = (N_LAYERS, USE_F32R)
    if key not in _CACHED:
        _CACHED[key] = build_program()
    return _CACHED[key]


def run_device(inputs, trace=False):
    shared, x0t = prep_arrays(inputs)
    nc = _get_program()
    in_maps = [dict(shared, x0t=x0t[b]) for b in range(B)]
    res = run_bass_kernel_spmd(nc, in_maps, core_ids=list(range(B)),
                               trace=trace)
    logits = np.stack([np.asarray(res.results[b]["logits"]) for b in range(B)])
    lse = np.stack([np.asarray(res.results[b]["lse"])[:, 0] for b in range(B)])
    return logits, lse, res


def kernel(**inputs):
    logits, lse, _ = run_device(inputs)
    tgt = np.asarray(inputs['targets'])
    tl = np.take_along_axis(logits, tgt[..., None].astype(np.int64), axis=-1)[..., 0]
    loss = np.mean(lse - tl, dtype=np.float64).astype(np.float32)
    return logits, loss
